# revision 2
# baseline (speedup 1.0000x reference)
"""Trainium2 Bass kernel for nn_ContourLoss (masked cosine-KNN retrieval loss).

Reference computation:
    emb   = train_data @ W + b                      (512, 2)
    probs = softmax(logits)                         (512, 16)
    pred  = argmax(probs, 1); ncls = argmax(all_nodes, 1)
    sim   = cos_sim(probs, all_nodes)               (512, 100000)
    closest = argmax_n where ncls[n]==pred[b] of sim[b, n]
    loss  = mean((emb - all_nodes_2d[closest])**2)

Device strategy (8 cores, nodes sharded 12500/core):
    For the argmax, row-positive scalings of probs are irrelevant, so each
    core scores u_b . vhat_n with u = exp(logits_row), vhat = node/||node||,
    plus an additive +4.0 class-match bump folded into the same K=32 matmul
    (rows 16..31 = 4*onehot(pred) x onehot(node_class)).  Per sample the
    masked argmax = plain argmax of the bumped scores.  Each core returns
    its local (max score, arg index); host reduces across the 8 cores,
    gathers all_nodes_2d, and computes the tiny final MSE.
"""

import numpy as np
from contextlib import ExitStack

import concourse.bass as bass
import concourse.bacc as bacc
import concourse.tile as tile
from concourse import mybir
from concourse.bass_utils import run_bass_kernel_spmd

dt = mybir.dt
AF = mybir.ActivationFunctionType
ALU = mybir.AluOpType
AX = mybir.AxisListType

B = 512          # batch
C = 16           # classes
N = 100000       # nodes
NCORES = 8
NSH = N // NCORES          # 12500 nodes per core
J = 100                    # free-dim groups per partition (128*100 = 12800)
NPAD = 128 * J             # padded shard size
CH = B // 128              # batch chunks of 128
TN = 512                   # matmul moving tile (one PSUM bank)
NT = NPAD // TN            # node tiles per chunk


def build_program():
    nc = bacc.Bacc("TRN2", target_bir_lowering=False, debug=False)
    f32 = dt.float32

    nodes_rows = nc.dram_tensor("nodes_rows", [128, J * 16], f32, kind="ExternalInput")
    logits_rows = nc.dram_tensor("logits_rows", [128, CH * 16], f32, kind="ExternalInput")
    logitsT = nc.dram_tensor("logitsT", [16, B], f32, kind="ExternalInput")
    out_max = nc.dram_tensor("out_max", [128, CH], f32, kind="ExternalOutput")
    out_idx = nc.dram_tensor("out_idx", [128, CH], dt.uint32, kind="ExternalOutput")

    rhat_dram = nc.dram_tensor("rhat_scr", [16, NPAD], f32)
    oh_dram = nc.dram_tensor("oh_scr", [16, NPAD], f32)
    pred_dram = nc.dram_tensor("pred_scr", [B], f32)

    with tile.TileContext(nc) as tc, ExitStack() as ctx:
        const = ctx.enter_context(tc.tile_pool(name="const", bufs=1))
        work = ctx.enter_context(tc.tile_pool(name="work", bufs=1))
        big = ctx.enter_context(tc.tile_pool(name="big", bufs=1))
        simp = ctx.enter_context(tc.tile_pool(name="simsp", bufs=1))
        psum = ctx.enter_context(tc.tile_pool(name="psum", bufs=4, space="PSUM"))

        # ---- loads ----
        nr = const.tile([128, J * 16], f32)
        nc.sync.dma_start(nr[:], nodes_rows.ap())
        lr = const.tile([128, CH * 16], f32)
        nc.sync.dma_start(lr[:], logits_rows.ap())
        lt = const.tile([16, B], f32)
        nc.sync.dma_start(lt[:], logitsT.ap())

        # ---- iotas ----
        iota16i = const.tile([128, 16], dt.int32)
        nc.gpsimd.iota(iota16i[:], pattern=[[1, 16]], base=0, channel_multiplier=0)
        iota16f = const.tile([128, 16], f32)
        nc.vector.tensor_copy(iota16f[:], iota16i[:])
        iotarev = const.tile([128, 16], f32)  # 16 - c
        nc.vector.tensor_scalar(iotarev[:], iota16f[:], -1.0, 16.0, op0=ALU.mult, op1=ALU.add)
        iotaP16i = const.tile([16, 1], dt.int32)  # partition index 0..15
        nc.gpsimd.iota(iotaP16i[:], pattern=[[0, 1]], base=0, channel_multiplier=1)
        iotaP16f = const.tile([16, 1], f32)
        nc.vector.tensor_copy(iotaP16f[:], iotaP16i[:])

        # ---- predicted class per sample (argmax over 16 logits) ----
        lr3 = lr[:].rearrange("p (a c) -> p a c", c=16)
        mx = work.tile([128, CH], f32)
        nc.vector.reduce_max(mx[:], lr3, axis=AX.X)
        eq = work.tile([128, CH * 16], f32)
        eq3 = eq[:].rearrange("p (a c) -> p a c", c=16)
        nc.vector.tensor_tensor(eq3, lr3, mx[:].unsqueeze(2).broadcast_to([128, CH, 16]), op=ALU.is_equal)
        ctr = work.tile([128, CH * 16], f32)
        ctr3 = ctr[:].rearrange("p (a c) -> p a c", c=16)
        nc.vector.tensor_tensor(ctr3, eq3, iotarev[:].unsqueeze(1).broadcast_to([128, CH, 16]), op=ALU.mult)
        mrev = work.tile([128, CH], f32)
        nc.vector.reduce_max(mrev[:], ctr3, axis=AX.X)
        pred = work.tile([128, CH], f32)
        nc.vector.tensor_scalar(pred[:], mrev[:], -1.0, 16.0, op0=ALU.mult, op1=ALU.add)
        # round-trip through DRAM to get pred in [16, B] transposed+replicated form
        nc.sync.dma_start(pred_dram.ap().rearrange("(a p) -> p a", a=CH), pred[:])
        predT = work.tile([16, B], f32)
        nc.sync.dma_start(predT[:], pred_dram.ap().unsqueeze(0).broadcast_to([16, B]))

        # ---- extended stationary matrix [32, B]: rows 0-15 exp(logits.T), rows 16-31 4*onehot(pred) ----
        bl = const.tile([32, B], f32)
        nc.scalar.activation(bl[0:16, :], lt[:], AF.Exp)
        maskT = work.tile([16, B], f32)
        nc.vector.tensor_scalar(maskT[:], predT[:], iotaP16f[:], 4.0, op0=ALU.is_equal, op1=ALU.mult)
        nc.sync.dma_start(bl[16:32, :], maskT[:])

        # ---- node preprocessing in natural layout [128, J, 16] ----
        nr3 = nr[:].rearrange("p (j c) -> p j c", c=16)
        sq = work.tile([128, J * 16], f32)
        sq3 = sq[:].rearrange("p (j c) -> p j c", c=16)
        nc.vector.tensor_tensor(sq3, nr3, nr3, op=ALU.mult)
        ss = work.tile([128, J], f32)
        nc.vector.reduce_sum(ss[:], sq3, axis=AX.X)
        normt = work.tile([128, J], f32)
        nc.scalar.activation(normt[:], ss[:], AF.Sqrt)
        z0 = work.tile([128, J], f32)
        nc.vector.reciprocal(z0[:], normt[:])
        # one Newton step: rn = z0 * (1.5 - 0.5*ss*z0^2)  -> ~1e-7 accurate rsqrt
        t0 = work.tile([128, J], f32)
        nc.vector.tensor_tensor(t0[:], z0[:], z0[:], op=ALU.mult)
        nc.vector.tensor_tensor(t0[:], t0[:], ss[:], op=ALU.mult)
        nc.vector.tensor_scalar(t0[:], t0[:], -0.5, 1.5, op0=ALU.mult, op1=ALU.add)
        rn = work.tile([128, J], f32)
        nc.vector.tensor_tensor(rn[:], z0[:], t0[:], op=ALU.mult)
        rhat = work.tile([128, J * 16], f32)
        rhat3 = rhat[:].rearrange("p (j c) -> p j c", c=16)
        nc.vector.tensor_tensor(rhat3, nr3, rn[:].unsqueeze(2).broadcast_to([128, J, 16]), op=ALU.mult)

        # node class argmax + onehot
        mxn = work.tile([128, J], f32)
        nc.vector.reduce_max(mxn[:], nr3, axis=AX.X)
        eqn = work.tile([128, J * 16], f32)
        eqn3 = eqn[:].rearrange("p (j c) -> p j c", c=16)
        nc.vector.tensor_tensor(eqn3, nr3, mxn[:].unsqueeze(2).broadcast_to([128, J, 16]), op=ALU.is_equal)
        ctn = work.tile([128, J * 16], f32)
        ctn3 = ctn[:].rearrange("p (j c) -> p j c", c=16)
        nc.vector.tensor_tensor(ctn3, eqn3, iotarev[:].unsqueeze(1).broadcast_to([128, J, 16]), op=ALU.mult)
        mrevn = work.tile([128, J], f32)
        nc.vector.reduce_max(mrevn[:], ctn3, axis=AX.X)
        clsn = work.tile([128, J], f32)
        nc.vector.tensor_scalar(clsn[:], mrevn[:], -1.0, 16.0, op0=ALU.mult, op1=ALU.add)
        ohn = work.tile([128, J * 16], f32)
        ohn3 = ohn[:].rearrange("p (j c) -> p j c", c=16)
        nc.vector.tensor_tensor(
            ohn3,
            iota16f[:].unsqueeze(1).broadcast_to([128, J, 16]),
            clsn[:].unsqueeze(2).broadcast_to([128, J, 16]),
            op=ALU.is_equal,
        )

        # ---- transpose via DRAM round-trip: [128, J, 16] -> [16, NPAD] ----
        nc.sync.dma_start(rhat_dram.ap().rearrange("c (p j) -> p j c", p=128), rhat3)
        nc.sync.dma_start(oh_dram.ap().rearrange("c (p j) -> p j c", p=128), ohn3)
        rhs = big.tile([32, NPAD], f32)
        nc.sync.dma_start(rhs[0:16, :], rhat_dram.ap())
        nc.sync.dma_start(rhs[16:32, :], oh_dram.ap())

        # ---- main: per batch chunk, matmul all node tiles then argmax ----
        for ch in range(CH):
            sims = simp.tile([128, NPAD], f32, tag="sims")
            for t in range(NT):
                ps = psum.tile([128, TN], f32, tag="ps")
                nc.tensor.matmul(
                    ps[:], bl[:, ch * 128:(ch + 1) * 128], rhs[:, t * TN:(t + 1) * TN],
                    start=True, stop=True,
                )
                nc.scalar.copy(sims[:, t * TN:(t + 1) * TN], ps[:])
            top8 = work.tile([128, 8], f32, tag="top8")
            idx8 = work.tile([128, 8], dt.uint32, tag="idx8")
            nc.vector.max(top8[:], sims[:])
            nc.vector.max_index(idx8[:], top8[:], sims[:])
            nc.sync.dma_start(out_max.ap()[:, ch:ch + 1], top8[:, 0:1])
            nc.sync.dma_start(out_idx.ap()[:, ch:ch + 1], idx8[:, 0:1])

    nc.compile()
    return nc


def make_in_maps(logits, all_nodes):
    logits = np.ascontiguousarray(logits, dtype=np.float32)
    all_nodes = np.ascontiguousarray(all_nodes, dtype=np.float32)
    logits_rows = np.ascontiguousarray(
        logits.reshape(CH, 128, 16).transpose(1, 0, 2).reshape(128, CH * 16))
    logitsT = np.ascontiguousarray(logits.T)
    in_maps = []
    for k in range(NCORES):
        shard = all_nodes[k * NSH:(k + 1) * NSH]
        pad = np.concatenate(
            [shard, np.broadcast_to(shard[0], (NPAD - NSH, 16))], axis=0)
        nodes_rows = np.ascontiguousarray(pad.reshape(128, J * 16))
        in_maps.append({
            "nodes_rows": nodes_rows,
            "logits_rows": logits_rows,
            "logitsT": logitsT,
        })
    return in_maps


def finalize(results, train_data, all_nodes_2d, W, b):
    """Cross-core argmax reduction + tiny final MSE (the unshard step)."""
    vals = np.stack([r["out_max"].T.reshape(B) for r in results])      # (8, B)
    locs = np.stack([r["out_idx"].T.reshape(B).astype(np.int64) for r in results])
    locs = np.where(locs < NSH, locs, 0)  # padded duplicate can only tie node 0
    gidx = locs + (np.arange(NCORES, dtype=np.int64) * NSH)[:, None]
    best_core = np.argmax(vals, axis=0)                               # first max wins
    closest = gidx[best_core, np.arange(B)]
    emb = train_data.astype(np.float32) @ W.astype(np.float32) + b.astype(np.float32)
    target = all_nodes_2d[closest]
    per_sample = np.mean((emb - target) ** 2, axis=1)
    return np.array(np.mean(per_sample), dtype=np.float32)


_PROG = None


def kernel(**inputs):
    global _PROG
    logits = np.asarray(inputs["logits"], np.float32)
    train_data = np.asarray(inputs["train_data"], np.float32)
    all_nodes = np.asarray(inputs["all_nodes"], np.float32)
    all_nodes_2d = np.asarray(inputs["all_nodes_2d"], np.float32)
    W = np.asarray(inputs["W"], np.float32)
    b = np.asarray(inputs["b"], np.float32)

    if _PROG is None:
        _PROG = build_program()
    in_maps = make_in_maps(logits, all_nodes)
    res = run_bass_kernel_spmd(_PROG, in_maps, list(range(NCORES)))
    return finalize(res.results, train_data, all_nodes_2d, W, b)
